# revision 17
# baseline (speedup 1.0000x reference)
"""CLUB loss kernel for Trainium2, 8 NeuronCores (SPMD data-parallel).

Math: with flat_x (N,d), iv = exp(-p_logvar):
  positive_i = -0.5 * sum_d (x_i - mu_i)^2 * iv_i
  negative_i = -0.5 * sum_d iv_i * (ex2 - 2 mu_i ex + mu_i^2)
  loss = mean_i(positive_i - negative_i)
Decomposed into global sums (single pass over data):
  sx[d], sxx[d], A[d]=sum iv, B2[d]=sum iv*mu, Ta=sum iv*x^2, Tb=sum iv*mu*x
  loss = -0.5/N * [(Ta - 2 Tb) - dot(sxx,A)/N + dot(sx,2*B2)/N]

v3 design (v2 measured 53.9us; trace breakdown: 8.8us fixed preamble to
first HBM byte, ~31us gapless DMA stream at line rate, ~9us compute tail
after the last byte, ~4.5us postamble):
 - The tail was the lever: v2's last tile was 2048 rows with lv arriving
   LAST, so exp->jj (GPSIMD, serial) gated the final M2 matmuls, the PE
   idled >3.4us and HAM re-throttled it to 1.2GHz right when it mattered.
 - v3 uses 9 groups (7x1024 + 2x512 rows). The last two groups DMA in
   mu,lv,x order so the x-chain (cast->transpose->square->matmul) is the
   only post-stream work, on a 512-row group: tail ~4us.
 - ~72 tiny warm-up matmuls run during the DMA-wait preamble (PE is idle
   there) so the HAM clock gate opens before real matmuls start, and the
   no-stall ordering keeps it open through the tail.
 - Wrap-up is spread across ACT (PSUM column copies) / DVE (diag
   extracts) / PE (partition fold) instead of a serial DVE chain.
Everything else keeps the v2 structure: contiguous DRAM loads (>=2KB per
partition chunk), i-major permuted mu/lv (partition p of group g holds
rows R*p..R*p+R-1), x PE-transposed per 128-col group with column stride
R reproducing the same permutation, coupled sums as PE matmuls with fp16
operands accumulated in fp32 PSUM over all 64 blocks:
  M1: lhsT=iv_blk,  rhs=[xsqT_blk | ones] -> diag = Ta partials, col128 = A
  M2: lhsT=jj_blk,  rhs=[xT_blk   | ones] -> diag = Tb partials, col128 = B2
  M3: lhsT=ones,    rhs=[xsqT_blk | ones] -> row = sxx partials
  M4: lhsT=ones,    rhs=[xT_blk   | ones] -> row = sx partials
Device emits per-core (128,4) + (2,128) stats; host does the O(d) combine.
"""

import numpy as np

B, D, H, W = 16, 128, 64, 64
N = B * H * W            # 65536
NCORES = 8
BPC = B // NCORES        # 2 batches per core
HW = H * W               # 4096
ROWS = BPC * HW          # 8192 rows per core
GROUPS = [1024] * 7 + [512] * 2   # rows per pipeline group (sum = ROWS)
NG = len(GROUPS)
GSTART = [sum(GROUPS[:i]) for i in range(NG)]
GBLK = [sum(GROUPS[:i]) // 128 for i in range(NG)]  # first global block
NBLK = ROWS // 128       # 64 blocks per core

_CACHE = {}


def _build_nc(stats_output=True):
    import concourse.bass as bass
    import concourse.bacc as bacc
    import concourse.mybir as mybir
    from concourse import masks
    from concourse.tile import TileContext

    f32 = mybir.dt.float32
    f16 = mybir.dt.float16
    ALU = mybir.AluOpType
    AF = mybir.ActivationFunctionType
    AX = mybir.AxisListType

    nc = bacc.Bacc(num_devices=NCORES)
    x_in = nc.dram_tensor("x", [BPC, D, HW], f32, kind="ExternalInput")
    mu_in = nc.dram_tensor("p_mu", [ROWS, D], f32, kind="ExternalInput")
    lv_in = nc.dram_tensor("p_logvar", [ROWS, D], f32, kind="ExternalInput")
    stats_out = nc.dram_tensor("stats", [128, 4], f32, kind="ExternalOutput")
    rows_out = nc.dram_tensor("rows", [1, 256], f32, kind="ExternalOutput")

    with TileContext(nc) as tc:
        with (
            tc.tile_pool(name="const", bufs=1) as constp,
            tc.tile_pool(name="slabs", bufs=NG) as slabs,
            tc.tile_pool(name="big", bufs=1) as big,
            tc.tile_pool(name="work", bufs=5) as work,
            tc.tile_pool(name="stats", bufs=1) as stats,
            tc.tile_pool(name="ps", bufs=4, space="PSUM") as psp,
            tc.tile_pool(name="psacc", bufs=1, space="PSUM") as psacc,
        ):
            # issue every input DMA first: the triggers have no deps, and
            # the HWDGE ring drains them FIFO at HBM line rate — the
            # earlier they start, the earlier the whole pipeline finishes.
            # Stream order: [x0,mu0,lv0] [mu7,lv7,mu8,lv8] [x1..lv6]
            # [x7] [x8] — the tail groups' mu/lv land by ~16us so their
            # exp+jj are long done when x7/x8 (the final bytes) arrive,
            # leaving only the short x-chain after the stream ends.
            slabs_xml = []
            dmas = {}
            for g, rows in enumerate(GROUPS):
                r0 = GSTART[g]
                b, hw0 = r0 // HW, r0 % HW
                x_t = slabs.tile([128, 1024], f32, tag="x_t", name="x_t")
                mu_t = slabs.tile([128, 1024], f32, tag="mu_t", name="mu_t")
                lv_t = slabs.tile([128, 1024], f32, tag="lv_t", name="lv_t")
                def dma_x(x_t=x_t, b=b, hw0=hw0, rows=rows):
                    nc.sync.dma_start(out=x_t[:, :rows],
                                      in_=x_in[b, :, hw0:hw0 + rows])
                def dma_mu(mu_t=mu_t, r0=r0, rows=rows):
                    nc.sync.dma_start(
                        out=mu_t[:, :rows],
                        in_=mu_in[r0:r0 + rows, :].rearrange(
                            "(p r) d -> p (r d)", p=128))
                def dma_lv(lv_t=lv_t, r0=r0, rows=rows):
                    nc.sync.dma_start(
                        out=lv_t[:, :rows],
                        in_=lv_in[r0:r0 + rows, :].rearrange(
                            "(p r) d -> p (r d)", p=128))
                dmas[g] = (dma_x, dma_mu, dma_lv)
                slabs_xml.append((x_t, mu_t, lv_t))
            dmas[0][0](); dmas[0][1](); dmas[0][2]()
            for g in (NG - 2, NG - 1):
                dmas[g][1](); dmas[g][2]()
            for g in range(1, NG - 2):
                dmas[g][0](); dmas[g][1](); dmas[g][2]()
            dmas[NG - 2][0]()
            dmas[NG - 1][0]()

            ident16 = constp.tile([128, 128], f16, name="ident16")
            masks.make_identity(nc, ident16[:])
            identf = constp.tile([128, 128], f32, name="identf")
            masks.make_identity(nc, identf[:])
            onecell = constp.tile([1, 1], f32, name="onecell")
            nc.vector.memset(onecell[:], 1.0)
            ones_col = constp.tile([128, 1], f16, name="ones_col")
            nc.vector.memset(ones_col[:], 1.0)

            # PSUM accumulators for the coupled matmul streams
            P1 = psacc.tile([128, 129], f32, name="P1")
            P2 = psacc.tile([128, 129], f32, name="P2")
            P34 = psacc.tile([1, 258], f32, name="P34")

            # HAM warm-up: ~72 tiny matmuls during the DMA-wait preamble
            # (PE idle 7-13us). ~3.6us of sustained PE activity opens the
            # clock gate (1.2 -> 2.4 GHz) before the real stream starts;
            # the dense stream + no-stall tail then keep it open. Results
            # land in P3, which the first real M3 (start=True) clears.
            for _ in range(100):
                nc.tensor.matmul(P34[0:1, 0:32], ones_col[:],
                                 ident16[:, 0:32], start=True, stop=True,
                                 skip_group_check=True)

            # persistent transposed-x layout, interleaved per block:
            # [xsqT(128) | ones | xT(128) | ones] = 258 cols. M1 reads
            # cols 0:129, M2 reads 129:258, and ONE ones-matmul (M34,
            # N=258) covers both sxx and sx — merging the two ones
            # streams halves their PE slot cost (LDW-bound ~107ns each).
            comb = big.tile([128, NBLK * 258], f16, name="comb")
            comb_v = comb[:].rearrange("p (n c) -> p n c", c=258)
            nc.vector.memset(comb_v[:, :, 128:129], 1.0)
            nc.vector.memset(comb_v[:, :, 257:258], 1.0)

            ivs, jjs = {}, {}

            def emit_mm(g):
                """M1/M2/M3/M4 matmuls for group g's blocks.

                For the last group, all M1s come first so P1 completes
                as early as possible and the DVE diag-extract overlaps
                the remaining M2/M3/M4 matmuls.
                """
                iv, jj = ivs[g], jjs[g]
                R = GROUPS[g] // 128
                def m1(r):
                    blk = GBLK[g] + r
                    nc.tensor.matmul(
                        P1[:], iv[:, r * 128:(r + 1) * 128],
                        comb_v[:, blk, 0:129], start=blk == 0,
                        stop=blk == NBLK - 1, skip_group_check=True)
                def m2(r):
                    blk = GBLK[g] + r
                    nc.tensor.matmul(
                        P2[:], jj[:, r * 128:(r + 1) * 128],
                        comb_v[:, blk, 129:258], start=blk == 0,
                        stop=blk == NBLK - 1, skip_group_check=True)
                def m34(r):
                    blk = GBLK[g] + r
                    nc.tensor.matmul(
                        P34[:], ones_col[:],
                        comb_v[:, blk, 0:258], start=blk == 0,
                        stop=blk == NBLK - 1, skip_group_check=True)
                if g == NG - 1:
                    for r in range(R):
                        m1(r)
                    for r in range(R):
                        m2(r)
                    for r in range(R):
                        m34(r)
                else:
                    for r in range(R):
                        m1(r); m2(r); m34(r)

            # bufs = NG: every iv/jj fully resident — zero ring reuse, so
            # no WAR coupling between late exps and earlier consumers
            for g in range(NG):
                ivs[g] = work.tile([128, 1024], f16, tag="iv", name="iv",
                                   bufs=NG)
                jjs[g] = work.tile([128, 1024], f16, tag="jj", name="jj",
                                   bufs=NG)

            def emit_exp_jj(g):
                iv, jj = ivs[g], jjs[g]
                _, mu_t, lv_t = slabs_xml[g]
                for c0 in range(0, GROUPS[g], 512):
                    sl = slice(c0, c0 + 512)
                    # ACT: iv = exp(-lv) (f32 -> f16), 512-col chunks
                    # so jj can chase the exp
                    nc.scalar.activation(iv[:, sl], lv_t[:, sl],
                                         AF.Exp, bias=0.0, scale=-1.0)
                    # GPSIMD: j = iv * mu (mixed f16*f32 -> f16; the
                    # DVE mixed path is a microcode disaster)
                    nc.gpsimd.tensor_tensor(jj[:, sl], iv[:, sl],
                                            mu_t[:, sl], ALU.mult)

            def emit_xside(g):
                """cast -> transposes -> copy/square for group g."""
                rows = GROUPS[g]
                R = rows // 128
                x_t = slabs_xml[g][0]
                xb = work.tile([128, 1024], f16, tag="xb", name="xb",
                               bufs=3)
                # ACT: xb = fp16(x); the transposes read stride-R
                # columns so they need the whole group cast
                nc.scalar.activation(xb[:, :rows], x_t[:, :rows],
                                     AF.Copy)
                # stride-R column view: xb_g[:, k, r] = xb[:, R*k + r],
                # so transpose block r puts row r0 + R*k + r on partition
                # k — exactly the mu/lv DMA permutation.
                xb_g = xb[:, :rows].rearrange("p (k s) -> p k s", s=R)
                psx = psp.tile([128, 1024], f16, tag="psx", name="psx")
                for r in range(R):
                    nc.tensor.transpose(psx[:, r * 128:(r + 1) * 128],
                                        xb_g[:, :, r], ident16[:])
                blk0 = GBLK[g]
                # DVE: plain copy into the ones-strided layout
                nc.vector.tensor_copy(
                    comb_v[:, blk0:blk0 + R, 129:257], psx[:, :R * 128])
                # DVE: square as xT(SBUF) * psx(PSUM) — keeps squares
                # off ACT and uses one read port per space
                nc.vector.tensor_tensor(
                    comb_v[:, blk0:blk0 + R, 0:128],
                    comb_v[:, blk0:blk0 + R, 129:257], psx[:, :R * 128],
                    ALU.mult)

            # program order mirrors arrival order: group 0 first, then
            # the tail groups' exp/jj (their mu/lv stream right after
            # group 0), then the bulk with a one-group matmul lag, then
            # the two tail x-chains
            emit_xside(0)
            emit_exp_jj(0)
            for g in (NG - 2, NG - 1):
                emit_exp_jj(g)
            for g in range(1, NG - 2):
                emit_exp_jj(g)
                emit_xside(g)
                emit_mm(g - 1)
            emit_xside(NG - 2)
            emit_mm(NG - 3)
            emit_xside(NG - 1)
            emit_mm(NG - 2)
            emit_mm(NG - 1)

            # ---- wrap-up: fold into gstat[128,4] + rows[2,128], spread
            # over ACT (PSUM column/row copies) and DVE (diag extracts)
            # so the serial chain after the last matmul stays short ----
            gstat = stats.tile([128, 4], f32, name="gstat")
            scratch = stats.tile([128, 128], f32, name="scratch")
            # A, B2 from the ones columns (ACT sits next to PSUM)
            nc.scalar.activation(gstat[:, 0:1], P1[:, 128:129], AF.Copy)
            nc.scalar.activation(gstat[:, 1:2], P2[:, 128:129], AF.Copy)
            # Ta, Tb from the diagonals (DVE)
            nc.vector.tensor_tensor(scratch[:], P1[:, 0:128], identf[:],
                                    ALU.mult)
            nc.vector.tensor_reduce(gstat[:, 2:3], scratch[:], axis=AX.X,
                                    op=ALU.add)
            nc.vector.tensor_tensor(scratch[:], P2[:, 0:128], identf[:],
                                    ALU.mult)
            nc.vector.tensor_reduce(gstat[:, 3:4], scratch[:], axis=AX.X,
                                    op=ALU.add)
            # sxx (P34 cols 0:128) and sx (P34 cols 129:257) rows go
            # out as-is; host reads them
            srow = stats.tile([1, 256], f32, name="srow")
            nc.scalar.activation(srow[0:1, 0:128], P34[0:1, 0:128],
                                 AF.Copy)
            nc.scalar.activation(srow[0:1, 128:256], P34[0:1, 129:257],
                                 AF.Copy)

            nc.sync.dma_start(out=rows_out[:], in_=srow[:])
            nc.sync.dma_start(out=stats_out[:], in_=gstat[:])

    return nc


MODE = "host"


def get_nc(use_collective=True, stats_output=True):
    key = ("nc_v8",)
    if key not in _CACHE:
        nc = _build_nc()
        if not nc.is_finalized():
            nc.finalize()
        _CACHE[key] = nc
    return _CACHE[key]


def make_in_maps(x, p_mu, p_logvar):
    x = np.ascontiguousarray(np.asarray(x, dtype=np.float32))
    p_mu = np.ascontiguousarray(np.asarray(p_mu, dtype=np.float32))
    p_logvar = np.ascontiguousarray(np.asarray(p_logvar, dtype=np.float32))
    in_maps = []
    for c in range(NCORES):
        in_maps.append({
            "x": np.ascontiguousarray(
                x[c * BPC:(c + 1) * BPC].reshape(BPC, D, HW)),
            "p_mu": np.ascontiguousarray(p_mu[c * ROWS:(c + 1) * ROWS]),
            "p_logvar": np.ascontiguousarray(
                p_logvar[c * ROWS:(c + 1) * ROWS]),
        })
    return in_maps


def kernel(x, p_mu, p_logvar):
    from concourse.bass_utils import run_bass_kernel_spmd

    in_maps = make_in_maps(x, p_mu, p_logvar)
    nc = get_nc()
    res = run_bass_kernel_spmd(nc, in_maps, list(range(NCORES)))
    s = np.zeros((128, 4), dtype=np.float64)
    rr = np.zeros((2, 128), dtype=np.float64)
    for c in range(NCORES):
        s += np.asarray(res.results[c]["stats"], dtype=np.float64)
        rr += np.asarray(res.results[c]["rows"],
                         dtype=np.float64).reshape(2, 128)
    A, B2p, Ta, Tb = (s[:, k] for k in range(4))
    sxx, sx = rr[0], rr[1]
    T = Ta.sum() - 2.0 * Tb.sum()
    loss = -0.5 / N * (T - sxx.dot(A) / N + sx.dot(2.0 * B2p) / N)
    return np.asarray(loss, dtype=np.float32).reshape(())
